# revision 16
# baseline (speedup 1.0000x reference)
"""MoE inter-cycle layer on 8 Trainium2 NeuronCores (Bass/Tile).

Strategy
--------
Host computes the gate matrix (softmax * mask, renormalized) from the tiny
[B, E] logits/masks and routes only ACTIVE (row, expert) pairs to the
device.  All 8 cores run one identical SPMD program with static addressing;
per-core differences are pure data:

* static slots: for each expert e, floor(R_e/8) pairs per core (rows dealt
  round-robin), weights DMA'd once per expert from the replicated [E,...]
  weight inputs — zero padding.
* flex slots: the leftover R_e mod 8 pairs across all experts are dealt to
  cores (ceil(total_leftover/8) slots per core); each flex slot's weights
  come from a small host-gathered per-core weight stream, so the slot's
  expert identity is data, not program structure.

Each slot computes gelu(x[b] @ w1[e] + b1[e]) @ w2[e] for the 512 tokens of
row b; the host combines per-slot outputs with the gate weights (plus the
gates @ b2 bias term), mirroring the reference's dispatch/combine.  Matmuls
run in float32r (full-rate fp32 on the PE array, ~2e-4 relative error),
with X pre-transposed on the host so both matmuls consume natural layouts.
"""

import numpy as np

import concourse.tile as tile
import concourse.mybir as mybir
from concourse import bacc
from concourse.bass_utils import run_bass_kernel_spmd
from concourse.tile_rust import add_dep_helper

B, L, D, F, E = 64, 512, 512, 2048, 8
N_CORES = 8
P = 128
EPS = 1e-9

# Set by kernel() after a run: BassKernelResults (exec_time_ns under BASS_TRACE=1)
LAST_RESULT = None


def _build_program(M, nflex):
    """One SPMD program. M[e] = static per-core slots for expert e,
    nflex = per-core flex slots (weights from the gathered flex stream)."""
    S = int(sum(M)) + nflex
    f32 = mybir.dt.float32
    f32r = mybir.dt.float32r

    nc = bacc.Bacc("TRN2", target_bir_lowering=False, debug=False,
                   num_devices=N_CORES)
    xt_d = nc.dram_tensor("xts", [S, D, L], f32r, kind="ExternalInput").ap()
    w1_d = nc.dram_tensor("w1r", [E, D, F], f32r, kind="ExternalInput").ap()
    w2_d = nc.dram_tensor("w2r", [E, F, D], f32r, kind="ExternalInput").ap()
    b1_d = nc.dram_tensor("b1f", [E, F], f32, kind="ExternalInput").ap()
    if nflex:
        fw1_d = nc.dram_tensor("fw1", [nflex, D, F], f32r,
                               kind="ExternalInput").ap()
        fw2_d = nc.dram_tensor("fw2", [nflex, F, D], f32r,
                               kind="ExternalInput").ap()
        fb1_d = nc.dram_tensor("fb1", [nflex, F], f32,
                               kind="ExternalInput").ap()
    y_d = nc.dram_tensor("ys", [S, L, D], f32, kind="ExternalOutput").ap()

    KD = D // P   # 4  k-chunks for matmul1
    KF = F // P   # 16 k-chunks for matmul2
    NT = L // P   # 4  token tiles for matmul2 output

    with tile.TileContext(nc) as tc:
        with (
            tc.tile_pool(name="wpool", bufs=2) as wpool,
            tc.tile_pool(name="bpool", bufs=2) as bpool,
            tc.tile_pool(name="xpool", bufs=3) as xpool,
            tc.tile_pool(name="gpool", bufs=1) as gpool,
            tc.tile_pool(name="opool", bufs=2) as opool,
            tc.tile_pool(name="psum", bufs=4, space="PSUM") as psum,
        ):
            slot = 0
            # first ACT of each weight-group's first slot; weight DMAs of
            # group i are gated on the previous group's anchor so startup
            # only races in the first group's w1 + xt0, not every prefetch
            first_act = [None]

            def gate(dma_inst, anchor):
                if anchor is not None:
                    add_dep_helper(dma_inst.ins, anchor.ins, sync=True,
                                   reason="stagger weight prefetch")

            def load_weights(w1_src, w2_src, b1_src):
                # per-K-chunk tiles: matmuls wait only on the chunk they
                # read, so loads pipeline into compute at group boundaries
                dmas = []
                w1_sb = []
                for ki in range(KD):
                    t = wpool.tile([P, F], f32r, tag=f"w1_{ki}")
                    dmas.append(nc.gpsimd.dma_start(
                        t[:], w1_src[ki * P:(ki + 1) * P, :]))
                    w1_sb.append(t)
                w2_sb = []
                w2_dmas = []
                for fj in range(KF):
                    t = wpool.tile([P, D], f32r, tag=f"w2_{fj}")
                    w2_dmas.append(nc.gpsimd.dma_start(
                        t[:], w2_src[fj * P:(fj + 1) * P, :]))
                    w2_sb.append(t)
                b1_sb = bpool.tile([P, KF], f32, tag="b1")
                dmas.append(nc.sync.dma_start(
                    b1_sb[:], b1_src.rearrange("(ko p) -> p ko", p=P)))
                return w1_sb, w2_sb, b1_sb, dmas, w2_dmas

            def pair_slot(w1_sb, w2_sb, b1_sb, slot, xt_anchor=None):
                xt_sb = xpool.tile([P, KD, L], f32r, tag="xt")
                gate(nc.sync.dma_start(
                    xt_sb[:],
                    xt_d[slot].rearrange("(ko p) t -> p ko t", p=P)),
                    xt_anchor)

                # matmul1 + gelu: G[f,t] = gelu(sum_d w1[d,f] xt[d,t] + b1[f])
                g_sb = gpool.tile([P, KF, L], f32r, tag="g")
                act0 = None
                for fi in range(KF):
                    ph = psum.tile([P, L], f32, tag="ph")
                    for ki in range(KD):
                        nc.tensor.matmul(
                            ph[:],
                            w1_sb[ki][:, fi * P:(fi + 1) * P],
                            xt_sb[:, ki, :],
                            start=(ki == 0),
                            stop=(ki == KD - 1),
                        )
                    act = nc.scalar.activation(
                        g_sb[:, fi, :], ph[:],
                        mybir.ActivationFunctionType.Gelu,
                        bias=b1_sb[:, fi:fi + 1],
                    )
                    if fi == 0:
                        act0 = act

                # matmul2: Y[t,d] = sum_f G[f,t] w2[f,d]
                for ti in range(NT):
                    py = psum.tile([P, D], f32, tag="py")
                    for fj in range(KF):
                        nc.tensor.matmul(
                            py[:],
                            g_sb[:, fj, ti * P:(ti + 1) * P],
                            w2_sb[fj][:],
                            start=(fj == 0),
                            stop=(fj == KF - 1),
                        )
                    y_sb = opool.tile([P, D], f32, tag="ysb")
                    nc.vector.tensor_copy(y_sb[:], py[:])
                    nc.sync.dma_start(
                        y_d[slot, ti * P:(ti + 1) * P, :], y_sb[:])
                return act0

            groups = [(w1_d[e], w2_d[e], b1_d[e], M[e])
                      for e in range(E) if M[e] > 0]
            groups += [(fw1_d[j], fw2_d[j], fb1_d[j], 1)
                       for j in range(nflex)]
            # only groups 0 and 1 can issue DMA at t=0 (wpool bufs=2 gates
            # later groups naturally), so only their loads need staggering
            # to keep the startup race down to group 0's w1 + xt0
            prev_anchor = None
            for gi, (w1_src, w2_src, b1_src, n_slots) in enumerate(groups):
                w1_sb, w2_sb, b1_sb, wdmas, w2dmas = load_weights(
                    w1_src, w2_src, b1_src)
                anchor = None
                for k in range(n_slots):
                    a = pair_slot(w1_sb, w2_sb, b1_sb, slot,
                                  xt_anchor=anchor if (gi == 0 and k > 0)
                                  else None)
                    if k == 0:
                        anchor = a
                    slot += 1
                if gi == 0:
                    # group 0's w2 is not read until its first matmul2
                    for dm in w2dmas:
                        gate(dm, anchor)
                elif gi == 1:
                    for dm in wdmas + w2dmas:
                        gate(dm, prev_anchor)
                prev_anchor = anchor
    nc.compile()
    return nc


def kernel(cycle_curve_data, logits, moe_masks, w1, b1, w2, b2):
    global LAST_RESULT
    x = np.asarray(cycle_curve_data, dtype=np.float32)
    logits = np.asarray(logits, dtype=np.float32)
    moe_masks = np.asarray(moe_masks)
    w1 = np.ascontiguousarray(np.asarray(w1, dtype=np.float32))
    b1 = np.ascontiguousarray(np.asarray(b1, dtype=np.float32))
    w2 = np.ascontiguousarray(np.asarray(w2, dtype=np.float32))
    b2 = np.asarray(b2, dtype=np.float32)

    # ---- gates (host, fp32 softmax like the reference) ----
    mask = (moe_masks == 1).astype(np.float32)
    z = logits - logits.max(axis=1, keepdims=True)
    ez = np.exp(z)
    raw = ez / ez.sum(axis=1, keepdims=True)                    # [B, E]
    gated = raw * mask
    gates = gated / (gated.sum(axis=1, keepdims=True) + EPS)    # [B, E]
    guide_loss = np.float32(
        (1.0 - np.float32(np.sum(raw * mask)) / np.float32(B)) ** 2)

    # ---- routing ----
    rows_per_exp = [np.nonzero(mask[:, e])[0] for e in range(E)]
    M = [len(r) // N_CORES for r in rows_per_exp]               # static slots
    leftovers = []                                              # (b, e) pairs
    for e in range(E):
        for b_idx in rows_per_exp[e][M[e] * N_CORES:]:
            leftovers.append((int(b_idx), e))
    nflex = int(np.ceil(len(leftovers) / N_CORES))
    S = int(sum(M)) + nflex
    if S == 0:
        out = np.broadcast_to(
            (gates @ b2)[:, None, :], (B, L, D)).astype(np.float32).copy()
        return out, guide_loss

    # slot table: per core, list of (b, e) with b = -1 for flex padding
    slot_tab = [[] for _ in range(N_CORES)]
    for e in range(E):
        rows = rows_per_exp[e]
        for c in range(N_CORES):
            for j in range(M[e]):
                slot_tab[c].append((int(rows[j * N_CORES + c]), e))
    flex_tab = [[] for _ in range(N_CORES)]                     # (b, e) | None
    for c in range(N_CORES):
        mine = leftovers[c::N_CORES]
        for j in range(nflex):
            entry = mine[j] if j < len(mine) else None
            flex_tab[c].append(entry)
            slot_tab[c].append(entry if entry is not None else (-1, 0))

    # ---- gather per-core inputs ----
    xT = np.ascontiguousarray(x.transpose(0, 2, 1))             # [B, D, L]
    in_maps = []
    for c in range(N_CORES):
        xts = np.empty((S, D, L), dtype=np.float32)
        for s, (b_idx, _e) in enumerate(slot_tab[c]):
            xts[s] = xT[b_idx if b_idx >= 0 else 0]
        m = {"xts": xts, "w1r": w1, "w2r": w2, "b1f": b1}
        if nflex:
            fw1 = np.zeros((nflex, D, F), dtype=np.float32)
            fw2 = np.zeros((nflex, F, D), dtype=np.float32)
            fb1 = np.zeros((nflex, F), dtype=np.float32)
            for j, entry in enumerate(flex_tab[c]):
                if entry is not None:
                    fw1[j] = w1[entry[1]]
                    fw2[j] = w2[entry[1]]
                    fb1[j] = b1[entry[1]]
            m.update({"fw1": fw1, "fw2": fw2, "fb1": fb1})
        in_maps.append(m)

    # ---- build + run ----
    nc = _build_program(M, nflex)
    res = run_bass_kernel_spmd(nc, in_maps, list(range(N_CORES)))
    LAST_RESULT = res

    # ---- combine on host: out[b] = sum gates[b,e] * Y + gates @ b2 ----
    out = np.zeros((B, L, D), dtype=np.float32)
    for c in range(N_CORES):
        ys = res.results[c]["ys"]
        for s, (b_idx, e) in enumerate(slot_tab[c]):
            if b_idx >= 0:
                out[b_idx] += gates[b_idx, e] * ys[s]
    out += (gates @ b2)[:, None, :]
    return out, guide_loss


# revision 17
# speedup vs baseline: 1.0079x; 1.0079x over previous
"""MoE inter-cycle layer on 8 Trainium2 NeuronCores (Bass/Tile).

Strategy
--------
Host computes the gate matrix (softmax * mask, renormalized) from the tiny
[B, E] logits/masks and routes only ACTIVE (row, expert) pairs to the
device.  All 8 cores run one identical SPMD program with static addressing;
per-core differences are pure data:

* static slots: for each expert e, floor(R_e/8) pairs per core (rows dealt
  round-robin), weights DMA'd once per expert from the replicated [E,...]
  weight inputs — zero padding.
* flex slots: the leftover R_e mod 8 pairs across all experts are dealt to
  cores (ceil(total_leftover/8) slots per core); each flex slot's weights
  come from a small host-gathered per-core weight stream, so the slot's
  expert identity is data, not program structure.

Each slot computes gelu(x[b] @ w1[e] + b1[e]) @ w2[e] for the 512 tokens of
row b; the host combines per-slot outputs with the gate weights (plus the
gates @ b2 bias term), mirroring the reference's dispatch/combine.  Matmuls
run in float32r (full-rate fp32 on the PE array, ~2e-4 relative error),
with X pre-transposed on the host so both matmuls consume natural layouts.
"""

import numpy as np

import concourse.tile as tile
import concourse.mybir as mybir
from concourse import bacc
from concourse.bass_utils import run_bass_kernel_spmd
from concourse.tile_rust import add_dep_helper

B, L, D, F, E = 64, 512, 512, 2048, 8
N_CORES = 8
P = 128
EPS = 1e-9

# Set by kernel() after a run: BassKernelResults (exec_time_ns under BASS_TRACE=1)
LAST_RESULT = None


def _build_program(M, nflex):
    """One SPMD program. M[e] = static per-core slots for expert e,
    nflex = per-core flex slots (weights from the gathered flex stream)."""
    S = int(sum(M)) + nflex
    f32 = mybir.dt.float32
    f32r = mybir.dt.float32r

    nc = bacc.Bacc("TRN2", target_bir_lowering=False, debug=False,
                   num_devices=N_CORES)
    xt_d = nc.dram_tensor("xts", [S, D, L], f32r, kind="ExternalInput").ap()
    w1_d = nc.dram_tensor("w1r", [E, D, F], f32r, kind="ExternalInput").ap()
    w2_d = nc.dram_tensor("w2r", [E, F, D], f32r, kind="ExternalInput").ap()
    b1_d = nc.dram_tensor("b1f", [E, F], f32, kind="ExternalInput").ap()
    if nflex:
        fw1_d = nc.dram_tensor("fw1", [nflex, D, F], f32r,
                               kind="ExternalInput").ap()
        fw2_d = nc.dram_tensor("fw2", [nflex, F, D], f32r,
                               kind="ExternalInput").ap()
        fb1_d = nc.dram_tensor("fb1", [nflex, F], f32,
                               kind="ExternalInput").ap()
    y_d = nc.dram_tensor("ys", [S, L, D], f32, kind="ExternalOutput").ap()

    KD = D // P   # 4  k-chunks for matmul1
    KF = F // P   # 16 k-chunks for matmul2
    NT = L // P   # 4  token tiles for matmul2 output

    with tile.TileContext(nc) as tc:
        with (
            tc.tile_pool(name="wpool", bufs=2) as wpool,
            tc.tile_pool(name="bpool", bufs=2) as bpool,
            tc.tile_pool(name="xpool", bufs=3) as xpool,
            tc.tile_pool(name="gpool", bufs=1) as gpool,
            tc.tile_pool(name="opool", bufs=2) as opool,
            tc.tile_pool(name="psum", bufs=4, space="PSUM") as psum,
        ):
            slot = 0
            # first ACT of each weight-group's first slot; weight DMAs of
            # group i are gated on the previous group's anchor so startup
            # only races in the first group's w1 + xt0, not every prefetch
            first_act = [None]

            def gate(dma_inst, anchor):
                if anchor is not None:
                    add_dep_helper(dma_inst.ins, anchor.ins, sync=True,
                                   reason="stagger weight prefetch")

            def load_weights(w1_src, w2_src, b1_src):
                # per-K-chunk tiles: matmuls wait only on the chunk they
                # read, so loads pipeline into compute at group boundaries
                dmas = []
                w1_sb = []
                for ki in range(KD):
                    t = wpool.tile([P, F], f32r, tag=f"w1_{ki}")
                    dmas.append(nc.sync.dma_start(
                        t[:], w1_src[ki * P:(ki + 1) * P, :]))
                    w1_sb.append(t)
                w2_sb = []
                w2_dmas = []
                for fj in range(KF):
                    t = wpool.tile([P, D], f32r, tag=f"w2_{fj}")
                    w2_dmas.append(nc.sync.dma_start(
                        t[:], w2_src[fj * P:(fj + 1) * P, :]))
                    w2_sb.append(t)
                b1_sb = bpool.tile([P, KF], f32, tag="b1")
                dmas.append(nc.sync.dma_start(
                    b1_sb[:], b1_src.rearrange("(ko p) -> p ko", p=P)))
                return w1_sb, w2_sb, b1_sb, dmas, w2_dmas

            def pair_slot(w1_sb, w2_sb, b1_sb, slot, xt_anchor=None):
                xt_sb = xpool.tile([P, KD, L], f32r, tag="xt")
                gate(nc.sync.dma_start(
                    xt_sb[:],
                    xt_d[slot].rearrange("(ko p) t -> p ko t", p=P)),
                    xt_anchor)

                # matmul1 + gelu: G[f,t] = gelu(sum_d w1[d,f] xt[d,t] + b1[f])
                g_sb = gpool.tile([P, KF, L], f32r, tag="g")
                act0 = None
                for fi in range(KF):
                    ph = psum.tile([P, L], f32, tag="ph")
                    for ki in range(KD):
                        nc.tensor.matmul(
                            ph[:],
                            w1_sb[ki][:, fi * P:(fi + 1) * P],
                            xt_sb[:, ki, :],
                            start=(ki == 0),
                            stop=(ki == KD - 1),
                        )
                    act = nc.scalar.activation(
                        g_sb[:, fi, :], ph[:],
                        mybir.ActivationFunctionType.Gelu,
                        bias=b1_sb[:, fi:fi + 1],
                    )
                    if fi == 0:
                        act0 = act

                # matmul2: Y[t,d] = sum_f G[f,t] w2[f,d]
                for ti in range(NT):
                    py = psum.tile([P, D], f32, tag="py")
                    for fj in range(KF):
                        nc.tensor.matmul(
                            py[:],
                            g_sb[:, fj, ti * P:(ti + 1) * P],
                            w2_sb[fj][:],
                            start=(fj == 0),
                            stop=(fj == KF - 1),
                        )
                    y_sb = opool.tile([P, D], f32, tag="ysb")
                    nc.vector.tensor_copy(y_sb[:], py[:])
                    nc.sync.dma_start(
                        y_d[slot, ti * P:(ti + 1) * P, :], y_sb[:])
                return act0

            groups = [(w1_d[e], w2_d[e], b1_d[e], M[e])
                      for e in range(E) if M[e] > 0]
            groups += [(fw1_d[j], fw2_d[j], fb1_d[j], 1)
                       for j in range(nflex)]
            # only groups 0 and 1 can issue DMA at t=0 (wpool bufs=2 gates
            # later groups naturally), so only their loads need staggering
            # to keep the startup race down to group 0's w1 + xt0
            prev_anchor = None
            for gi, (w1_src, w2_src, b1_src, n_slots) in enumerate(groups):
                w1_sb, w2_sb, b1_sb, wdmas, w2dmas = load_weights(
                    w1_src, w2_src, b1_src)
                anchor = None
                for k in range(n_slots):
                    a = pair_slot(w1_sb, w2_sb, b1_sb, slot,
                                  xt_anchor=anchor if (gi == 0 and k > 0)
                                  else None)
                    if k == 0:
                        anchor = a
                    slot += 1
                if gi == 0:
                    # group 0's w2 is not read until its first matmul2
                    for dm in w2dmas:
                        gate(dm, anchor)
                elif gi == 1:
                    for dm in wdmas + w2dmas:
                        gate(dm, prev_anchor)
                prev_anchor = anchor
    nc.compile()
    return nc


def kernel(cycle_curve_data, logits, moe_masks, w1, b1, w2, b2):
    global LAST_RESULT
    x = np.asarray(cycle_curve_data, dtype=np.float32)
    logits = np.asarray(logits, dtype=np.float32)
    moe_masks = np.asarray(moe_masks)
    w1 = np.ascontiguousarray(np.asarray(w1, dtype=np.float32))
    b1 = np.ascontiguousarray(np.asarray(b1, dtype=np.float32))
    w2 = np.ascontiguousarray(np.asarray(w2, dtype=np.float32))
    b2 = np.asarray(b2, dtype=np.float32)

    # ---- gates (host, fp32 softmax like the reference) ----
    mask = (moe_masks == 1).astype(np.float32)
    z = logits - logits.max(axis=1, keepdims=True)
    ez = np.exp(z)
    raw = ez / ez.sum(axis=1, keepdims=True)                    # [B, E]
    gated = raw * mask
    gates = gated / (gated.sum(axis=1, keepdims=True) + EPS)    # [B, E]
    guide_loss = np.float32(
        (1.0 - np.float32(np.sum(raw * mask)) / np.float32(B)) ** 2)

    # ---- routing ----
    rows_per_exp = [np.nonzero(mask[:, e])[0] for e in range(E)]
    M = [len(r) // N_CORES for r in rows_per_exp]               # static slots
    leftovers = []                                              # (b, e) pairs
    for e in range(E):
        for b_idx in rows_per_exp[e][M[e] * N_CORES:]:
            leftovers.append((int(b_idx), e))
    nflex = int(np.ceil(len(leftovers) / N_CORES))
    S = int(sum(M)) + nflex
    if S == 0:
        out = np.broadcast_to(
            (gates @ b2)[:, None, :], (B, L, D)).astype(np.float32).copy()
        return out, guide_loss

    # slot table: per core, list of (b, e) with b = -1 for flex padding
    slot_tab = [[] for _ in range(N_CORES)]
    for e in range(E):
        rows = rows_per_exp[e]
        for c in range(N_CORES):
            for j in range(M[e]):
                slot_tab[c].append((int(rows[j * N_CORES + c]), e))
    flex_tab = [[] for _ in range(N_CORES)]                     # (b, e) | None
    for c in range(N_CORES):
        mine = leftovers[c::N_CORES]
        for j in range(nflex):
            entry = mine[j] if j < len(mine) else None
            flex_tab[c].append(entry)
            slot_tab[c].append(entry if entry is not None else (-1, 0))

    # ---- gather per-core inputs ----
    xT = np.ascontiguousarray(x.transpose(0, 2, 1))             # [B, D, L]
    in_maps = []
    for c in range(N_CORES):
        xts = np.empty((S, D, L), dtype=np.float32)
        for s, (b_idx, _e) in enumerate(slot_tab[c]):
            xts[s] = xT[b_idx if b_idx >= 0 else 0]
        m = {"xts": xts, "w1r": w1, "w2r": w2, "b1f": b1}
        if nflex:
            fw1 = np.zeros((nflex, D, F), dtype=np.float32)
            fw2 = np.zeros((nflex, F, D), dtype=np.float32)
            fb1 = np.zeros((nflex, F), dtype=np.float32)
            for j, entry in enumerate(flex_tab[c]):
                if entry is not None:
                    fw1[j] = w1[entry[1]]
                    fw2[j] = w2[entry[1]]
                    fb1[j] = b1[entry[1]]
            m.update({"fw1": fw1, "fw2": fw2, "fb1": fb1})
        in_maps.append(m)

    # ---- build + run ----
    nc = _build_program(M, nflex)
    res = run_bass_kernel_spmd(nc, in_maps, list(range(N_CORES)))
    LAST_RESULT = res

    # ---- combine on host: out[b] = sum gates[b,e] * Y + gates @ b2 ----
    out = np.zeros((B, L, D), dtype=np.float32)
    for c in range(N_CORES):
        ys = res.results[c]["ys"]
        for s, (b_idx, e) in enumerate(slot_tab[c]):
            if b_idx >= 0:
                out[b_idx] += gates[b_idx, e] * ys[s]
    out += (gates @ b2)[:, None, :]
    return out, guide_loss
